# revision 20
# baseline (speedup 1.0000x reference)
"""MultiHeadAttention (B=2, S=2048, D=1024, H=16) on 8 trn2 cores.

Tensor-parallel over heads: core c owns heads 2c, 2c+1 (128 output features).
Per core:
  phase A: project q/k/v transposed:  qT = (Wq_c/8) @ X^T   [128 feat, 4096 tok]
           (X^T streamed from DRAM in bf16; W shards resident)
           v additionally PE-transposed to natural layout and augmented with a
           ones column per 128-token chunk (rowsum rides the attn@v matmul).
  phase B: per (batch, head):  S^T = kT^T-chunks @ qT  (scores transposed:
           key-tokens on partitions, query-tokens free)  ->  exp on ScalarE
           -> multiply by notmask (bf16, DVE) -> out^T[65, n] += v_aug^T @ expT
           accumulated over key chunks in PSUM.  out^T row 64 is the softmax
           denominator.  Division happens on host in fp32.

Schedule (v2): the kernel front is DMA-bound (panels 24MB + mask 16MB must
stream at ~0.37MB/us), so:
  - x panels are host-pre-shuffled to [128, ncb, kc, 512] so one 1024-token
    pair panel is a single DMA with 16KB-contiguous per-partition descriptors.
  - mask chunks ride the GpSimd SWDGE ring; panels ride the sync HWDGE ring,
    so the two streams don't head-of-line block each other.
  - minimal front: k0 (2 pairs), v0 first pair (+transpose), q0 first pair,
    then attention on batch 0 starts; everything else (v0/q0 second pairs,
    all batch-1 projections) is interleaved at ~1 kc-step (2 matmuls) per
    attention tile, with slots chosen to meet consume deadlines.
  - PE warm-up runs on a memset tile (no DMA dependency) so the HAM clock
    ramps while the first panels stream; exp's ACT table preloads at t=0.
  - projections run as 1024-token pairs accumulating in one [128,1024] psum
    tile (2 banks): halves weight-load count and drain instruction count.
PSUM: proj 2 banks (bufs=1) + scores 4 (bufs=2) + out 2 (bufs=1) = 8.
"""

import sys

sys.path.insert(0, "/opt/trn_rl_repo")

import numpy as np
import ml_dtypes

import concourse.mybir as mybir
import concourse.tile as tile
from concourse import bacc
from concourse.bass_utils import run_bass_kernel_spmd
from concourse.masks import make_identity

BF16 = mybir.dt.bfloat16
F32 = mybir.dt.float32
NP_BF16 = ml_dtypes.bfloat16

NCORES = 8
B, S, D = 2, 2048, 1024
H, DH = 16, 64
HPC = H // NCORES  # heads per core = 2
MPC = HPC * DH  # output features per core = 128
T = B * S  # 4096 tokens
NKC = D // 128  # 8 contraction chunks for projections
NNC = T // 512  # 8 token chunks of 512
NPAIR = 2  # token pairs (1024 tokens) per batch
NJC = S // 128  # 16 key-token chunks per batch
NTC = T // 128  # 32 global token chunks (v_aug)
VW = DH + 1  # 65: head dim + ones column

_CACHE: dict = {}


def _emit(nc, dins, dout):
    from contextlib import ExitStack

    tc = dins["_tc"]
    IDENT = mybir.ActivationFunctionType.Identity
    EXP = mybir.ActivationFunctionType.Exp
    with ExitStack() as ctx:
        singles = ctx.enter_context(tc.tile_pool(name="singles", bufs=1))

        # packed weights/biases: one DMA each so the first panel's ring slot
        # comes right after (ring is FIFO; every trigger ahead delays it)
        wsb = singles.tile([128, 3, NKC, 128], BF16, tag="wsb")
        bsb = singles.tile([128, 3], F32, tag="bsb")
        TI = {"q": 0, "k": 1, "v": 2}
        w_sb = {t: wsb[:, TI[t]] for t in ("q", "k", "v")}
        b_sb = {t: bsb[:, TI[t] : TI[t] + 1] for t in ("q", "k", "v")}

        qT = singles.tile([128, T], BF16, tag="qT")
        vT = singles.tile([128, T], BF16, tag="vT")
        # Packed kT: head h's 64 rows live at partitions h*64..h*64+64.
        kTp = singles.tile([128, T], BF16, tag="kTp")

        v_aug = [
            singles.tile([128, NTC * VW], BF16, tag=f"vaug{h}", name=f"vaug{h}")
            for h in range(HPC)
        ]

        ident = singles.tile([128, 128], BF16, tag="ident")
        # Warm-up tile: memset (no DMA dep) so PE can ramp from t~0.
        warm = singles.tile([128, 128], BF16, tag="warm")
        dummy_act = singles.tile([128, 1], BF16, tag="dummy_act")

        # mask chunk-pairs: rotating pool of 8 [128, 2, S] tiles (64KB/part).
        # b0 groups fill all 8 slots; b1 group g's DMA then WARs b0 group g's
        # last reader (attn0 tile 48+2g) -- self-pacing that also keeps its
        # ring slot behind the b1 panel triggers.
        nmp = ctx.enter_context(tc.tile_pool(name="nmp", bufs=8))
        nm_map = {}

        xp = ctx.enter_context(tc.tile_pool(name="xpanels", bufs=4))
        expp = ctx.enter_context(tc.tile_pool(name="expp", bufs=6))
        outsb = ctx.enter_context(tc.tile_pool(name="outsb", bufs=2))
        psP = ctx.enter_context(tc.tile_pool(name="psP", bufs=1, space="PSUM"))
        psS = ctx.enter_context(tc.tile_pool(name="psS", bufs=2, space="PSUM"))
        psO = ctx.enter_context(tc.tile_pool(name="psO", bufs=1, space="PSUM"))

        def emit_setup():
            nc.vector.memset(warm, 0.03125)
            for h in range(HPC):
                nc.vector.memset(v_aug[h], 1.0)
            # preload the exp spline tables while the front streams
            nc.scalar.activation(out=dummy_act, in_=warm[:, 0:1], func=EXP)
            make_identity(nc, ident)
            nc.sync.dma_start(out=wsb, in_=dins["wP"].ap())
            nc.sync.dma_start(out=bsb, in_=dins["bP"].ap())

        def warm_mms(n, use_po=False):
            # fillers between front pairs target the (idle) psO bank so they
            # don't WAR-stall on the proj psum slot's pending drain
            if use_po:
                ps = psO.tile([VW, 1024], F32, tag="out", name="warmpo")
                tgt, lhs = ps[:, :128], warm[:, :VW]
            else:
                ps = psP.tile([128, 1024], F32, tag="proj", name="warmps")
                tgt, lhs = ps[:, :128], warm
            for _ in range(n):
                nc.tensor.matmul(tgt, lhsT=lhs, rhs=warm, start=True, stop=True)

        # ---- projection pairs (1024 tokens each) --------------------------
        def pair_issue(t, b, pcb):
            def f():
                xt = xp.tile([128, 2, NKC, 512], BF16, tag="xpair", name="xpair")
                s0 = b * 2 * NPAIR + 2 * pcb
                nc.sync.dma_start(
                    out=xt, in_=dins[f"x{t}P"].ap()[:, s0 : s0 + 2]
                )
                st[(t, b, pcb, "xt")] = xt
            return f

        st = {}

        def pair_step(t, b, pcb, kc, borrow=None):
            def f():
                key = (t, b, pcb, "ps")
                if kc == 0:
                    pool = borrow if borrow is not None else psP
                    tag = "scores" if pool is psS else "proj"
                    st[key] = pool.tile([128, 1024], F32, tag=tag, name="projps")
                ps = st[key]
                xt = st[(t, b, pcb, "xt")]
                for h2 in range(2):
                    nc.tensor.matmul(
                        ps[:, h2 * 512 : (h2 + 1) * 512],
                        lhsT=w_sb[t][:, kc, :],
                        rhs=xt[:, h2, kc, :],
                        start=(kc == 0),
                        stop=(kc == NKC - 1),
                    )
            return f

        def pair_drain(t, b, pcb, dve):
            def f():
                ps = st.pop((t, b, pcb, "ps"))
                del st[(t, b, pcb, "xt")]
                col = b * S + pcb * 1024
                dst = {"q": qT, "k": kTp, "v": vT}[t]
                if dve:
                    nc.vector.tensor_add(
                        dst[:, col : col + 1024],
                        ps,
                        b_sb[t].broadcast_to([128, 1024]),
                    )
                else:
                    nc.scalar.activation(
                        out=dst[:, col : col + 1024], in_=ps, func=IDENT,
                        bias=b_sb[t],
                    )
            return f

        def vtr_step(b, ncb):
            """PE-transpose 512 projected v columns into v_aug."""
            def f():
                tbase = b * NJC + ncb * 4
                pst = psP.tile([128, 512], BF16, tag="proj", name="vtps")
                for i in range(4):
                    nc.tensor.transpose(
                        out=pst[:, i * 128 : (i + 1) * 128],
                        in_=vT[:, (tbase + i) * 128 : (tbase + i + 1) * 128],
                        identity=ident,
                    )
                for h in range(HPC):
                    src = pst.rearrange("p (i d) -> p i d", i=4)[
                        :, :, h * DH : (h + 1) * DH
                    ]
                    dst = v_aug[h][:, tbase * VW : (tbase + 4) * VW].rearrange(
                        "p (i w) -> p i w", i=4
                    )[:, :, 0:DH]
                    nc.vector.tensor_copy(out=dst, in_=src)
            return f

        def mask_issue(b, g):
            # One DMA per 2-chunk group (c = 2g, 2g+1).  All input DMAs ride
            # the sync ring: the scheduler hoists dependency-free triggers,
            # but ring transfers complete in trigger order, so emission order
            # IS the arrival priority; slot WARs (xpair pool, nm pool) hold
            # later triggers back to the right ring position.
            def f():
                t_ = nmp.tile([128, 2, S], BF16, tag="nm", name=f"nm{b}_{g}")
                nm_map[(b, 2 * g)] = (t_, 0)
                nm_map[(b, 2 * g + 1)] = (t_, 1)
                nc.sync.dma_start(
                    out=t_,
                    in_=dins["nmT"]
                    .ap()[b, 2 * g * 128 : (2 * g + 2) * 128, :]
                    .rearrange("(i p) s -> p i s", p=128),
                )
            return f

        def run_units(units):
            for u in units:
                u()

        def emit_drain(outps, b, h, nh, last=False):
            # drain in pieces so each store DMA starts as soon as its piece
            # is copied (short kernel tail).  Stores go on the gpsimd queue:
            # a sync-queue dma_start would head-of-line-block later panel
            # descriptors.
            osb = outsb.tile([VW, 1024], F32, tag="osb", name="osb")
            n = 4 if last else 2
            w = 1024 // n
            for s2 in range(n):
                nc.vector.tensor_copy(
                    out=osb[:, s2 * w : (s2 + 1) * w],
                    in_=outps[:, s2 * w : (s2 + 1) * w],
                )
                nc.gpsimd.dma_start(
                    out=dout.ap()[
                        b, h, :, nh * 1024 + s2 * w : nh * 1024 + (s2 + 1) * w
                    ],
                    in_=osb[:, s2 * w : (s2 + 1) * w],
                )

        def emit_attn(b, hooks, last=False):
            """Attention for batch b; hooks[tile_i] -> closures run before
            that jc-tile's emission.  attnv is software-pipelined one tile
            behind scores so the PE has queued work while tile j's
            exp->mask chain completes.  A group's psO drain is emitted
            lazily after the NEXT group's first exp/mask."""
            tile_i = -1
            pending = None
            attnv_q = None  # deferred attnv closure for the previous tile
            for nh in range(2):
                nbase = b * S + nh * 1024
                for h in range(HPC):
                    outps = None
                    for jc in range(NJC):
                        tile_i += 1
                        for fn in hooks.pop(tile_i, ()):
                            fn()
                        tglob = b * NJC + jc
                        ps = psS.tile([128, 1024], F32, tag="scores", name="scps")
                        for s2 in range(2):
                            nc.tensor.matmul(
                                ps[:, s2 * 512 : (s2 + 1) * 512],
                                lhsT=kTp[
                                    h * DH : (h + 1) * DH,
                                    tglob * 128 : (tglob + 1) * 128,
                                ],
                                rhs=qT[
                                    h * DH : (h + 1) * DH,
                                    nbase + s2 * 512 : nbase + (s2 + 1) * 512,
                                ],
                                start=True,
                                stop=True,
                            )
                        et = expp.tile([128, 1024], BF16, tag="exp", name="et")
                        nc.scalar.activation(out=et, in_=ps, func=EXP)
                        nmt, nmi = nm_map[(b, jc)]
                        nc.vector.tensor_mul(
                            et, et, nmt[:, nmi, nh * 1024 : (nh + 1) * 1024]
                        )
                        if attnv_q is not None:
                            attnv_q()
                        if jc == 0:
                            if pending is not None:
                                emit_drain(*pending)
                                pending = None
                            outps = psO.tile(
                                [VW, 1024], F32, tag="out", name="outps"
                            )

                        def mk_attnv(outps=outps, h=h, tglob=tglob, et=et, jc=jc):
                            def f():
                                for s2 in range(2):
                                    nc.tensor.matmul(
                                        outps[:, s2 * 512 : (s2 + 1) * 512],
                                        lhsT=v_aug[h][
                                            :, tglob * VW : tglob * VW + VW
                                        ],
                                        rhs=et[:, s2 * 512 : (s2 + 1) * 512],
                                        start=(jc == 0),
                                        stop=(jc == NJC - 1),
                                    )
                            return f

                        attnv_q = mk_attnv()
                    # flush the group's last attnv before leaving the group
                    attnv_q()
                    attnv_q = None
                    pending = (outps, b, h, nh)
            for ti in sorted(hooks):
                for fn in hooks.pop(ti):
                    fn()
            emit_drain(*pending, last=last)

        def spread(hooks, units, t0, per_tile=1):
            """Place units into hooks at per_tile units per tile from t0."""
            i = 0
            while i < len(units):
                hooks.setdefault(t0, []).extend(units[i : i + per_tile])
                i += per_tile
                t0 += 1
            return t0

        for _ in range(dins.get("_repeat", 1)):
            emit_setup()
            # ring trigger/arrival order: w, b, k00, k01, q00, v00, mg0,
            # v01, mg1..mg7, then slot-WAR-gated b1 panels, then b1 masks
            pair_issue("k", 0, 0)()
            pair_issue("k", 0, 1)()
            pair_issue("q", 0, 0)()
            warm_mms(22)
            # --- front: k0 fully, q0 first pair, v0 first pair (+vtr) ---
            run_units([pair_step("k", 0, 0, kc) for kc in range(NKC)])
            pair_issue("v", 0, 0)()
            mask_issue(0, 0)()
            pair_drain("k", 0, 0, dve=False)()
            warm_mms(8, use_po=True)
            run_units(
                [pair_step("k", 0, 1, kc, borrow=psS) for kc in range(NKC)]
            )
            pair_issue("v", 0, 1)()
            mask_issue(0, 1)()
            mask_issue(0, 2)()
            pair_drain("k", 0, 1, dve=False)()
            warm_mms(8, use_po=True)
            run_units([pair_step("q", 0, 0, kc) for kc in range(NKC)])
            pair_drain("q", 0, 0, dve=False)()
            mask_issue(0, 3)()
            mask_issue(0, 4)()
            warm_mms(8, use_po=True)
            run_units(
                [pair_step("v", 0, 0, kc, borrow=psS) for kc in range(NKC)]
            )
            pair_drain("v", 0, 0, dve=False)()
            mask_issue(0, 5)()
            mask_issue(0, 6)()
            mask_issue(0, 7)()
            vtr_step(0, 0)()
            vtr_step(0, 1)()

            # --- attn0 hooks: batch-0 leftovers + all batch-1 projections ---
            hooks = {}
            # v0 pair 1 (tglob 8-15, needed from tile 8; panel lands ~g0+v01
            # ring position)
            spread(
                hooks,
                [pair_step("v", 0, 1, kc) for kc in range(NKC)],
                3,
                per_tile=2,
            )
            hooks.setdefault(7, []).append(pair_drain("v", 0, 1, dve=True))
            hooks.setdefault(7, []).append(vtr_step(0, 2))
            hooks.setdefault(8, []).append(vtr_step(0, 3))
            # q0 pair 1 (nh1): needed at tile 32
            hooks.setdefault(6, []).append(pair_issue("q", 0, 1))
            t_ = spread(hooks, [pair_step("q", 0, 1, kc) for kc in range(NKC)], 9)
            hooks.setdefault(t_, []).append(pair_drain("q", 0, 1, dve=True))
            # batch-1 projections: everything attn1 tile 0 needs (kTp, qT
            # nh0, v_aug tglob 16-31) finishes before attn0's last tiles
            hooks.setdefault(13, []).append(pair_issue("k", 1, 0))
            t_ = spread(hooks, [pair_step("k", 1, 0, kc) for kc in range(NKC)], 16)
            hooks.setdefault(t_, []).append(pair_drain("k", 1, 0, dve=True))
            hooks.setdefault(21, []).append(pair_issue("k", 1, 1))
            t_ = spread(hooks, [pair_step("k", 1, 1, kc) for kc in range(NKC)], 25)
            hooks.setdefault(t_, []).append(pair_drain("k", 1, 1, dve=True))
            hooks.setdefault(30, []).append(pair_issue("q", 1, 0))
            t_ = spread(hooks, [pair_step("q", 1, 0, kc) for kc in range(NKC)], 34)
            hooks.setdefault(t_, []).append(pair_drain("q", 1, 0, dve=True))
            hooks.setdefault(39, []).append(pair_issue("v", 1, 0))
            t_ = spread(hooks, [pair_step("v", 1, 0, kc) for kc in range(NKC)], 43)
            hooks.setdefault(t_, []).append(pair_drain("v", 1, 0, dve=True))
            hooks.setdefault(52, []).append(vtr_step(1, 0))
            hooks.setdefault(53, []).append(vtr_step(1, 1))
            hooks.setdefault(48, []).append(pair_issue("v", 1, 1))
            t_ = spread(hooks, [pair_step("v", 1, 1, kc) for kc in range(NKC)], 54)
            hooks.setdefault(t_, []).append(pair_drain("v", 1, 1, dve=True))
            hooks.setdefault(62, []).append(vtr_step(1, 2))
            hooks.setdefault(63, []).append(vtr_step(1, 3))
            # masks b1 groups: the pool WAR (b0 group g's last read, attn0
            # tile 48+2g) holds each trigger to the right ring position
            spread(hooks, [mask_issue(1, g) for g in range(8)], 33)
            emit_attn(0, hooks)

            # --- attn1: q1 pair 1 (needed at tile 32) after the group-start
            # pipeline refill ---
            hooks = {}
            hooks.setdefault(14, []).append(pair_issue("q", 1, 1))
            t_ = spread(hooks, [pair_step("q", 1, 1, kc) for kc in range(NKC)], 17)
            hooks.setdefault(t_, []).append(pair_drain("q", 1, 1, dve=True))
            emit_attn(1, hooks, last=True)


def _build(repeat=1):
    key = ("nc", repeat)
    if key in _CACHE:
        return _CACHE[key]
    nc = bacc.Bacc("TRN2", target_bir_lowering=False, debug=False)
    dins = {}
    for t in ("q", "k", "v"):
        dins[f"x{t}P"] = nc.dram_tensor(
            f"x{t}P", [128, NNC, NKC, 512], BF16, kind="ExternalInput"
        )
    dins["wP"] = nc.dram_tensor("wP", [128, 3, NKC, 128], BF16, kind="ExternalInput")
    dins["bP"] = nc.dram_tensor("bP", [128, 3], F32, kind="ExternalInput")
    dins["nmT"] = nc.dram_tensor("nmT", [B, S, S], BF16, kind="ExternalInput")
    dout = nc.dram_tensor("out", [B, HPC, VW, S], F32, kind="ExternalOutput")

    with tile.TileContext(nc) as tc:
        dins["_tc"] = tc
        dins["_repeat"] = repeat
        _emit(nc, dins, dout)
        del dins["_tc"], dins["_repeat"]
    nc.compile()
    _CACHE[key] = nc
    return nc


def _prep_inputs(query, key, value, mask, Wq, bq, Wk, bk, Wv, bv):
    """Host-side shard prep. Returns per-core input maps."""
    xs = {}
    for name, x in (("q", query), ("k", key), ("v", value)):
        xt = np.asarray(x, dtype=np.float32).reshape(T, D).T  # [D, T]
        # [128, NNC, NKC, 512]: per-partition-contiguous pair panels
        xp_ = np.ascontiguousarray(
            xt.reshape(NKC, 128, NNC, 512).transpose(1, 2, 0, 3)
        ).astype(NP_BF16)
        xs[f"x{name}P"] = xp_

    nm = (~np.asarray(mask)).astype(NP_BF16)
    nmT = np.ascontiguousarray(np.transpose(nm, (0, 2, 1)))

    Wq = np.asarray(Wq, dtype=np.float32)
    Wk = np.asarray(Wk, dtype=np.float32)
    Wv = np.asarray(Wv, dtype=np.float32)
    bq = np.asarray(bq, dtype=np.float32)
    bk = np.asarray(bk, dtype=np.float32)
    bv = np.asarray(bv, dtype=np.float32)
    scale = 1.0 / np.sqrt(np.float32(DH))

    def wprep(wt):
        # wt: [D, MPC] -> [128, NKC, 128] with [p, kc, m] = wt[kc*128+p, m]
        return np.ascontiguousarray(
            wt.reshape(NKC, 128, MPC).transpose(1, 0, 2)
        ).astype(NP_BF16)

    in_maps = []
    for c in range(NCORES):
        r = slice(c * MPC, (c + 1) * MPC)
        m = dict(xs)
        m["nmT"] = nmT
        m["wP"] = np.ascontiguousarray(
            np.stack(
                [
                    wprep((Wq[r] * scale).T),
                    wprep(Wk[r].T),
                    wprep(Wv[r].T),
                ],
                axis=1,
            )
        )
        m["bP"] = np.ascontiguousarray(
            np.stack([bq[r] * scale, bk[r], bv[r]], axis=1).astype(np.float32)
        )
        in_maps.append(m)
    return in_maps


def _assemble(results):
    """results: per-core dicts with 'out' [B, HPC, 65, S] f32 -> [B, S, D]."""
    full = np.empty((B, S, D), dtype=np.float32)
    for c in range(NCORES):
        o = results[c]["out"]
        for b in range(B):
            for h in range(HPC):
                num = o[b, h, :DH, :]  # [64, S]
                den = o[b, h, DH, :]  # [S]
                col = c * MPC + h * DH
                full[b, :, col : col + DH] = (num / den).T
    return full


def kernel(query, key, value, mask, Wq, bq, Wk, bk, Wv, bv, **extra):
    nc = _build()
    in_maps = _prep_inputs(query, key, value, mask, Wq, bq, Wk, bk, Wv, bv)
    res = run_bass_kernel_spmd(nc, in_maps, core_ids=list(range(NCORES)))
    return _assemble(res.results)


def run_traced(inputs, **trace_kwargs):
    """For test.py: run with NTFF tracing, return (output, BassKernelResults)."""
    nc = _build()
    in_maps = _prep_inputs(**{k: inputs[k] for k in (
        "query", "key", "value", "mask", "Wq", "bq", "Wk", "bk", "Wv", "bv")})
    try:
        res = run_bass_kernel_spmd(
            nc, in_maps, core_ids=list(range(NCORES)), trace=True, **trace_kwargs
        )
    except ModuleNotFoundError:
        res = run_bass_kernel_spmd(nc, in_maps, core_ids=list(range(NCORES)))
    return _assemble(res.results), res
